# revision 1
# baseline (speedup 1.0000x reference)
"""Causal self-attention (QKV proj + RoPE + causal SDPA + out proj) on 8 trn2 cores.

Sharding: tensor-parallel over heads. Each core owns 2 of 16 heads:
  - Wqkv column-split (the core's q/k/v head rows), Wproj row-split.
  - Each core computes a full-shape partial of the output projection;
    the 8 partials are summed (and transposed back) on the host.

Device-side layout trick: everything runs transposed. The host feeds
x^T [C, B*T]; the QKV projection computes qkv^T = Wslice @ x with the
head dim on partitions, which is exactly what Q@K^T and the output
projection want as inputs, so no on-chip transposes are needed except
V (done with DMA xbar transposes, off the critical engines).
"""
import sys

sys.path.insert(0, "/opt/trn_rl_repo")

import numpy as np
import ml_dtypes

import concourse.bacc as bacc
import concourse.mybir as mybir
import concourse.tile as tile
from concourse.bass_utils import run_bass_kernel_spmd

N_CORES = 8
C = 2048
H = 16
D = 128
HPC = H // N_CORES          # heads per core = 2
PB = 512                    # row panel width
JB = 128                    # key tile width
NEG = -1.0e30
ROPE_BASE = 10000.0

BF = mybir.dt.bfloat16
F32 = mybir.dt.float32


def build_module(B, T):
    BT = B * T
    CC = C // 128            # contraction chunks for the projection
    FT = 3 * HPC             # qkv f-tiles per core (q0 q1 k0 k1 v0 v1)
    NPB = T // PB            # panels per batch
    NOC = C // 128           # out-proj column tiles
    scale = 1.0 / float(np.sqrt(D))

    nc = bacc.Bacc("TRN2", target_bir_lowering=False, debug=False,
                   num_devices=N_CORES)

    # x pre-tiled on host: xtiles[g, p, cc*PB + r] = x[g*PB + r, cc*128 + p]
    # -> one DMA per panel with 16KB contiguous runs (descriptor-rate bound
    #    HWDGE moves ~74GB/s at 1KB runs; long runs unlock full bandwidth)
    xtiles = nc.dram_tensor("xtiles", [BT // PB, 128, CC * PB], BF,
                            kind="ExternalInput").ap()
    wqkvT = nc.dram_tensor("wqkvT", [C, FT * 128], BF, kind="ExternalInput").ap()
    wprojT = nc.dram_tensor("wprojT", [HPC * 128, C], BF, kind="ExternalInput").ap()
    cosT = nc.dram_tensor("cosT", [128, T], BF, kind="ExternalInput").ap()
    sinT = nc.dram_tensor("sinT", [128, T], F32, kind="ExternalInput").ap()
    maskT = nc.dram_tensor("maskT", [128, 896], F32, kind="ExternalInput").ap()
    permT = nc.dram_tensor("permT", [128, 128], BF, kind="ExternalInput").ap()
    zout = nc.dram_tensor("zout", [C, BT], BF, kind="ExternalOutput").ap()

    with tile.TileContext(nc) as tc:
        with tc.tile_pool(name="sb", bufs=1) as sb, \
             tc.tile_pool(name="ps", bufs=1, space="PSUM") as ps:
            # ---- resident constants ----
            wqkv_sb = sb.tile([128, CC, FT * 128], BF, tag="wqkv", bufs=1)
            nc.sync.dma_start(
                out=wqkv_sb[:],
                in_=wqkvT.rearrange("(cc p) f -> p cc f", p=128))
            wproj_sb = sb.tile([128, HPC, C], BF, tag="wproj", bufs=1)
            nc.sync.dma_start(
                out=wproj_sb[:],
                in_=wprojT.rearrange("(hh p) o -> p hh o", p=128))
            cos_sb = sb.tile([128, T], BF, tag="cos", bufs=1)
            nc.sync.dma_start(out=cos_sb[:], in_=cosT)
            sin_sb = sb.tile([128, T], F32, tag="sin", bufs=1)
            nc.sync.dma_start(out=sin_sb[:], in_=sinT)
            mask_sb = sb.tile([128, 896], F32, tag="mask", bufs=1)
            nc.sync.dma_start(out=mask_sb[:], in_=maskT)
            perm_sb = sb.tile([128, 128], BF, tag="perm", bufs=1)
            nc.sync.dma_start(out=perm_sb[:], in_=permT)
            ones_col = sb.tile([128, 1], BF, tag="ones_c", bufs=1)
            nc.vector.memset(ones_col[:], 1.0)
            ones_row = sb.tile([1, 128], BF, tag="ones_r", bufs=1)
            nc.vector.memset(ones_row[:], 1.0)


            def emit_outproj(ypair, b, pp):
                r0g = b * T + pp * PB
                for oc in range(NOC):
                    zps = ps.tile([128, PB], F32, tag="mm", bufs=4)
                    for hh in range(HPC):
                        nc.tensor.matmul(
                            zps[:],
                            lhsT=wproj_sb[:, hh, oc * 128:(oc + 1) * 128],
                            rhs=ypair[hh][:],
                            start=(hh == 0), stop=(hh == HPC - 1))
                    zst = sb.tile([128, PB], BF, tag="zst", bufs=4)
                    nc.vector.tensor_copy(out=zst[:], in_=zps[:])
                    nc.gpsimd.dma_start(
                        out=zout[oc * 128:(oc + 1) * 128, r0g:r0g + PB],
                        in_=zst[:])

            HC = CC // 2

            def load_xt(b, pp):
                g = b * NPB + pp
                xta = sb.tile([128, HC, PB], BF, tag="xta", bufs=3,
                              name=f"xta_{b}_{pp}")
                xtb = sb.tile([128, HC, PB], BF, tag="xtb", bufs=3,
                              name=f"xtb_{b}_{pp}")
                src = xtiles[g].rearrange("p (cc r) -> p cc r", r=PB)
                nc.sync.dma_start(out=xta[:], in_=src[:, :HC, :])
                nc.gpsimd.dma_start(out=xtb[:], in_=src[:, HC:, :])
                return (xta, xtb)

            pending = None
            panels = [(b, pp) for b in range(B) for pp in range(NPB)]
            xt_q = [load_xt(*panels[0])]
            if len(panels) > 1:
                xt_q.append(load_xt(*panels[1]))
            gidx = 0
            for b in range(B):
                # ---------- projection + rope for batch b ----------
                q_t = [sb.tile([128, T], BF, tag=f"q{h}", bufs=2,
                               name=f"q{h}_{b}")
                       for h in range(HPC)]
                k_t = [sb.tile([128, T], BF, tag=f"k{h}", bufs=2,
                               name=f"k{h}_{b}")
                       for h in range(HPC)]
                v_t = [sb.tile([128, T // 128, 128], BF, tag=f"v{h}", bufs=2,
                               name=f"v{h}_{b}")
                       for h in range(HPC)]
                for pp in range(NPB):
                    r0g = b * T + pp * PB
                    ts = slice(pp * PB, pp * PB + PB)
                    xt = xt_q.pop(0)
                    if gidx + 2 < len(panels):
                        xt_q.append(load_xt(*panels[gidx + 2]))
                    gidx += 1
                    for ft in range(FT):
                        pps = ps.tile([128, PB], F32, tag="mm", bufs=4)
                        for cc in range(CC):
                            xsrc = xt[0][:, cc, :] if cc < HC \
                                else xt[1][:, cc - HC, :]
                            nc.tensor.matmul(
                                pps[:],
                                lhsT=wqkv_sb[:, cc, ft * 128:(ft + 1) * 128],
                                rhs=xsrc,
                                start=(cc == 0), stop=(cc == CC - 1))
                        if ft < 2 * HPC:   # q or k: apply rope
                            raw = sb.tile([128, PB], BF, tag="qkraw", bufs=2)
                            nc.scalar.copy(out=raw[:], in_=pps[:])
                            rot = ps.tile([128, PB], F32, tag="mm", bufs=4)
                            nc.tensor.matmul(rot[:], lhsT=perm_sb[:],
                                             rhs=raw[:], start=True, stop=True)
                            t1 = sb.tile([128, PB], F32, tag="t1", bufs=2)
                            nc.vector.tensor_mul(out=t1[:], in0=raw[:],
                                                 in1=cos_sb[:, ts])
                            t2 = sb.tile([128, PB], F32, tag="t2", bufs=2)
                            nc.vector.tensor_mul(out=t2[:], in0=rot[:],
                                                 in1=sin_sb[:, ts])
                            dest = (q_t if ft < HPC else k_t)[ft % HPC]
                            nc.vector.tensor_add(out=dest[:, ts], in0=t1[:],
                                                 in1=t2[:])
                        else:              # v: stage + dma-transpose
                            h = ft - 2 * HPC
                            vst = sb.tile([128, PB], BF, tag="vstage", bufs=2)
                            nc.scalar.copy(out=vst[:], in_=pps[:])
                            teng = nc.scalar
                            for q4 in range(PB // 128):
                                jt = pp * (PB // 128) + q4
                                teng.dma_start_transpose(
                                    out=v_t[h][:, jt, :],
                                    in_=vst[:, q4 * 128:(q4 + 1) * 128])
                    if pp == 0 and pending is not None:
                        emit_outproj(*pending)
                        pending = None
                # ---------- attention + out-proj for batch b ----------
                for pp in range(NPB):
                    nj = (pp + 1) * (PB // JB)
                    q0 = pp * PB
                    ytil = [ps.tile([128, PB], F32, tag="ytil", bufs=2,
                                    name=f"ytil{h}_{b}_{pp}")
                            for h in range(HPC)]
                    denom = [ps.tile([1, PB], F32, tag="small", bufs=2,
                                     name=f"den{h}_{b}_{pp}")
                             for h in range(HPC)]

                    def emit_S(h, j):
                        kk = j - pp * (PB // JB)
                        lo = max(kk, 0) * 128   # columns < lo fully masked
                        sps = ps.tile([128, PB], F32, tag="mm", bufs=4,
                                      name=f"s{h}_{b}_{pp}_{j}")
                        nc.tensor.matmul(
                            sps[:, lo:PB],
                            lhsT=k_t[h][:, j * JB:(j + 1) * JB],
                            rhs=q_t[h][:, q0 + lo:q0 + PB],
                            start=True, stop=True)
                        return sps

                    def emit_rest(h, j, sps):
                        kk = j - pp * (PB // JB)
                        lo = max(kk, 0) * 128
                        e = sb.tile([128, PB], BF, tag="e", bufs=4,
                                    name=f"e{h}_{b}_{pp}_{j}")
                        if kk >= 0:
                            # triangular 128-col slice gets the mask; the
                            # rest of the block is fully valid
                            nc.vector.scalar_tensor_tensor(
                                out=sps[:, lo:lo + 128],
                                in0=sps[:, lo:lo + 128], scalar=scale,
                                in1=mask_sb[:, 384:512],
                                op0=mybir.AluOpType.mult,
                                op1=mybir.AluOpType.add)
                            nc.scalar.activation(
                                out=e[:, lo:lo + 128], in_=sps[:, lo:lo + 128],
                                func=mybir.ActivationFunctionType.Exp)
                            if lo + 128 < PB:
                                nc.scalar.activation(
                                    out=e[:, lo + 128:PB],
                                    in_=sps[:, lo + 128:PB],
                                    func=mybir.ActivationFunctionType.Exp,
                                    scale=scale)
                        else:
                            nc.scalar.activation(
                                out=e[:, lo:PB], in_=sps[:, lo:PB],
                                func=mybir.ActivationFunctionType.Exp,
                                scale=scale)
                        nc.tensor.matmul(denom[h][:, lo:PB], lhsT=ones_col[:],
                                         rhs=e[:, lo:PB], start=(j == 0),
                                         stop=(j == nj - 1))
                        nc.tensor.matmul(ytil[h][:, lo:PB],
                                         lhsT=v_t[h][:, j, :],
                                         rhs=e[:, lo:PB], start=(j == 0),
                                         stop=(j == nj - 1))

                    jobs = [(h, j) for j in range(nj) for h in range(HPC)]
                    spss = {jobs[0]: emit_S(*jobs[0]),
                            jobs[1]: emit_S(*jobs[1])}
                    for idx, (h, j) in enumerate(jobs):
                        if idx + 2 < len(jobs):
                            spss[jobs[idx + 2]] = emit_S(*jobs[idx + 2])
                        emit_rest(h, j, spss.pop((h, j)))

                    ypair = []
                    for h in range(HPC):
                        dbf = sb.tile([1, PB], BF, tag="dbf", bufs=2)
                        nc.scalar.copy(out=dbf[:], in_=denom[h][:])
                        bc = ps.tile([128, PB], F32, tag="small", bufs=2,
                                     name=f"bc{h}_{b}_{pp}")
                        nc.tensor.matmul(bc[:], lhsT=ones_row[:],
                                         rhs=dbf[:], start=True, stop=True)
                        rec = sb.tile([128, PB], F32, tag="rec", bufs=2)
                        nc.vector.reciprocal_approx_fast(out=rec[:], in_=bc[:])
                        yp = sb.tile([128, PB], BF, tag="yp", bufs=6)
                        nc.vector.tensor_mul(out=yp[:], in0=ytil[h][:],
                                             in1=rec[:])
                        ypair.append(yp)
                    if pending is not None:
                        emit_outproj(*pending)
                    pending = (ypair, b, pp)
            emit_outproj(*pending)

    nc.compile()
    return nc


_module_cache = {}


def _get_module(B, T):
    key = (B, T)
    if key not in _module_cache:
        _module_cache[key] = build_module(B, T)
    return _module_cache[key]


def _host_prep(x, Wqkv, Wproj, B, T):
    bf16 = ml_dtypes.bfloat16
    BT = B * T
    NP = BT // PB
    CC = C // 128
    x2 = x.reshape(NP, PB, CC, 128)
    xtiles = np.ascontiguousarray(
        x2.transpose(0, 3, 2, 1).reshape(NP, 128, CC * PB)).astype(bf16)

    inv = 1.0 / (ROPE_BASE ** (np.arange(0, D, 2, dtype=np.float32) / D))
    t = np.arange(T, dtype=np.float32)
    fr = np.outer(t, inv)                      # [T, 64]
    emb = np.concatenate([fr, fr], -1)         # [T, 128]
    cosT = np.ascontiguousarray(np.cos(emb).T).astype(bf16)
    sinT = np.ascontiguousarray(np.sin(emb).T).astype(np.float32)

    g = np.arange(896)[None, :]
    p = np.arange(128)[:, None]
    maskT = np.where(g >= p + 384, 0.0, NEG).astype(np.float32)

    permT = np.zeros((128, 128), np.float32)
    for j in range(64):
        permT[j, j + 64] = 1.0                 # rot[i] = q[i-64] for i>=64
    for j in range(64, 128):
        permT[j, j - 64] = -1.0                # rot[i] = -q[i+64] for i<64
    permT = permT.astype(bf16)

    in_maps = []
    for c in range(N_CORES):
        heads = [HPC * c + h for h in range(HPC)]
        rows = []
        for blk in range(3):                   # q, k, v blocks of Wqkv
            for h in heads:
                r0 = blk * C + h * D
                rows.append(Wqkv[r0:r0 + D])
        wslice = np.concatenate(rows, 0)       # [FT*128, C]
        wqkvT = np.ascontiguousarray(wslice.T).astype(bf16)
        cols = np.concatenate([np.arange(h * D, (h + 1) * D) for h in heads])
        wprojT = np.ascontiguousarray(Wproj[:, cols].T).astype(bf16)
        in_maps.append({
            "xtiles": xtiles,
            "wqkvT": wqkvT,
            "wprojT": wprojT,
            "cosT": cosT,
            "sinT": sinT,
            "maskT": maskT,
            "permT": permT,
        })
    return in_maps


last_results = None


def kernel(x, Wqkv, Wproj, _trace=False, _trace_kwargs=None):
    global last_results
    x = np.asarray(x, dtype=np.float32)
    Wqkv = np.asarray(Wqkv, dtype=np.float32)
    Wproj = np.asarray(Wproj, dtype=np.float32)
    B, T, _C = x.shape
    assert _C == C and T % PB == 0

    nc = _get_module(B, T)
    in_maps = _host_prep(x, Wqkv, Wproj, B, T)
    res = run_bass_kernel_spmd(nc, in_maps, core_ids=list(range(N_CORES)),
                               trace=_trace, **(_trace_kwargs or {}))
    last_results = res
    z = res.results[0]["zout"].astype(np.float32)
    for c in range(1, N_CORES):
        z += res.results[c]["zout"].astype(np.float32)
    y = np.ascontiguousarray(z.T).reshape(B, T, C)
    return y



# revision 4
# speedup vs baseline: 1.2481x; 1.2481x over previous
"""Causal self-attention (QKV proj + RoPE + causal SDPA + out proj) on 8 trn2 cores.

Sharding: tensor-parallel over heads. Each core owns 2 of 16 heads:
  - Wqkv column-split (the core's q/k/v head rows), Wproj row-split.
  - Each core computes a full-shape partial of the output projection;
    the 8 partials are summed (and transposed back) on the host.

Device-side layout: everything runs transposed (x^T fed as [C, B*T];
qkv^T = W @ x keeps head dims on partitions). v is transposed on-chip
with DMA xbar transposes.

Schedule: batch b's attention+outproj is woven with batch b+1's
projection so the tensor engine never starves on the exp (scalar
engine) dependency chain. Softmax denominators are accumulated on
GpSimd (f32) instead of per-tile ones-matmuls on the PE.
"""
import sys

sys.path.insert(0, "/opt/trn_rl_repo")

import numpy as np
import ml_dtypes

import concourse.bacc as bacc
import concourse.mybir as mybir
import concourse.tile as tile
from concourse.bass_utils import run_bass_kernel_spmd

N_CORES = 8
C = 2048
H = 16
D = 128
HPC = H // N_CORES          # heads per core = 2
PB = 512                    # row panel width
JB = 128                    # key tile width
NEG = -1.0e30
ROPE_BASE = 10000.0

BF = mybir.dt.bfloat16
F32 = mybir.dt.float32


def build_module(B, T):
    BT = B * T
    CC = C // 128            # contraction chunks for the projection
    FT = 3 * HPC             # qkv f-tiles per core (q0 q1 k0 k1 v0 v1)
    NPB = T // PB            # panels per batch
    NOC = C // 128           # out-proj column tiles
    NPANEL = B * NPB
    scale = 1.0 / float(np.sqrt(D))
    AluAdd = mybir.AluOpType.add

    nc = bacc.Bacc("TRN2", target_bir_lowering=False, debug=False,
                   num_devices=N_CORES)

    # x pre-tiled on host: xtiles[g, p, cc*PB + r] = x[g*PB + r, cc*128 + p]
    xtiles = nc.dram_tensor("xtiles", [NPANEL, 128, CC * PB], BF,
                            kind="ExternalInput").ap()
    wqkvT = nc.dram_tensor("wqkvT", [C, FT * 128], BF, kind="ExternalInput").ap()
    wprojT = nc.dram_tensor("wprojT", [HPC * 128, C], BF, kind="ExternalInput").ap()
    cosT = nc.dram_tensor("cosT", [128, T], BF, kind="ExternalInput").ap()
    sinT = nc.dram_tensor("sinT", [128, T], F32, kind="ExternalInput").ap()
    maskT = nc.dram_tensor("maskT", [128, 896], F32, kind="ExternalInput").ap()
    permT = nc.dram_tensor("permT", [128, 128], BF, kind="ExternalInput").ap()
    # tiled output: zout[g, p, oc, r] = z[oc*128 + p, g*PB + r]
    # -> each panel store is one fully-contiguous 2MB DMA (16KB runs)
    zout = nc.dram_tensor("zout", [NPANEL, 128, NOC, PB], BF,
                          kind="ExternalOutput").ap()

    with tile.TileContext(nc) as tc:
        with tc.tile_pool(name="sb", bufs=1) as sb, \
             tc.tile_pool(name="ps", bufs=1, space="PSUM") as ps:
            # ---- resident constants ----
            wqkv_sb = sb.tile([128, CC, FT * 128], BF, tag="wqkv", bufs=1)
            nc.sync.dma_start(
                out=wqkv_sb[:],
                in_=wqkvT.rearrange("(cc p) f -> p cc f", p=128))
            wproj_sb = sb.tile([128, HPC, C], BF, tag="wproj", bufs=1)
            nc.sync.dma_start(
                out=wproj_sb[:],
                in_=wprojT.rearrange("(hh p) o -> p hh o", p=128))
            cos_sb = sb.tile([128, T], BF, tag="cos", bufs=1)
            nc.sync.dma_start(out=cos_sb[:], in_=cosT)
            sin_sb = sb.tile([128, T], F32, tag="sin", bufs=1)
            nc.sync.dma_start(out=sin_sb[:], in_=sinT)
            mask_sb = sb.tile([128, 896], F32, tag="mask", bufs=1)
            nc.sync.dma_start(out=mask_sb[:], in_=maskT)
            perm_sb = sb.tile([128, 128], BF, tag="perm", bufs=1)
            nc.sync.dma_start(out=perm_sb[:], in_=permT)
            ones_col = sb.tile([128, 1], BF, tag="ones_c", bufs=1)
            nc.vector.memset(ones_col[:], 1.0)
            ones_row = sb.tile([1, 128], BF, tag="ones_r", bufs=1)
            nc.vector.memset(ones_row[:], 1.0)

            HC = CC // 2
            xt_map = {}          # panel g -> (xta, xtb)
            bt_map = {}          # batch b -> dict(q=,k=,v=)

            def load_xt(g):
                xta = sb.tile([128, HC, PB], BF, tag="xta", bufs=2,
                              name=f"xta_{g}")
                xtb = sb.tile([128, HC, PB], BF, tag="xtb", bufs=2,
                              name=f"xtb_{g}")
                src = xtiles[g].rearrange("p (cc r) -> p cc r", r=PB)
                nc.sync.dma_start(out=xta[:], in_=src[:, :HC, :])
                nc.gpsimd.dma_start(out=xtb[:], in_=src[:, HC:, :])
                xt_map[g] = (xta, xtb)

            def batch_tiles(b):
                if b not in bt_map:
                    bt_map[b] = {
                        "q": [sb.tile([128, T], BF, tag=f"q{h}", bufs=2,
                                      name=f"q{h}_{b}") for h in range(HPC)],
                        "k": [sb.tile([128, T], BF, tag=f"k{h}", bufs=2,
                                      name=f"k{h}_{b}") for h in range(HPC)],
                        "v": [sb.tile([128, T // 128, 128], BF, tag=f"v{h}",
                                      bufs=2, name=f"v{h}_{b}")
                              for h in range(HPC)],
                    }
                return bt_map[b]

            # ---------------- projection (filler stream) ----------------
            # per panel: pending raw/vst tiles for the finalize unit
            proj_state = {}

            def emit_proj_group(b, pp, ft):
                g = b * NPB + pp
                if ft == 0:
                    if g + 1 < NPANEL:
                        load_xt(g + 1)
                    proj_state[(b, pp)] = {"raw": {}, "vst": {}}
                xt = xt_map[g]
                pps = ps.tile([128, PB], F32, tag="ps512", bufs=6)
                for cc in range(CC):
                    xsrc = xt[0][:, cc, :] if cc < HC else xt[1][:, cc - HC, :]
                    nc.tensor.matmul(
                        pps[:],
                        lhsT=wqkv_sb[:, cc, ft * 128:(ft + 1) * 128],
                        rhs=xsrc,
                        start=(cc == 0), stop=(cc == CC - 1))
                st = proj_state[(b, pp)]
                if ft < 2 * HPC:       # q or k: stage raw for rope
                    raw = sb.tile([128, PB], BF, tag="qkraw", bufs=4,
                                  name=f"raw_{b}_{pp}_{ft}")
                    nc.scalar.copy(out=raw[:], in_=pps[:])
                    st["raw"][ft] = raw
                else:                  # v: stage for dma-transpose
                    vst = sb.tile([128, PB], BF, tag="vstage", bufs=2,
                                  name=f"vst_{b}_{pp}_{ft}")
                    nc.scalar.copy(out=vst[:], in_=pps[:])
                    st["vst"][ft - 2 * HPC] = vst

            def emit_proj_finalize(b, pp):
                ts = slice(pp * PB, pp * PB + PB)
                tiles = batch_tiles(b)
                st = proj_state.pop((b, pp))
                rots = {}
                for ft in range(2 * HPC):     # rope rotate matmuls
                    rot = ps.tile([128, PB], F32, tag="ps512", bufs=6)
                    nc.tensor.matmul(rot[:], lhsT=perm_sb[:],
                                     rhs=st["raw"][ft][:], start=True,
                                     stop=True)
                    rots[ft] = rot
                for ft in range(2 * HPC):     # rope elementwise on DVE
                    raw = st["raw"][ft]
                    t1 = sb.tile([128, PB], BF, tag="t1", bufs=2)
                    nc.vector.tensor_mul(out=t1[:], in0=raw[:],
                                         in1=cos_sb[:, ts])
                    t2 = sb.tile([128, PB], BF, tag="t2", bufs=2)
                    nc.vector.tensor_mul(out=t2[:], in0=rots[ft][:],
                                         in1=sin_sb[:, ts])
                    dest = (tiles["q"] if ft < HPC else tiles["k"])[ft % HPC]
                    nc.vector.tensor_add(out=dest[:, ts], in0=t1[:],
                                         in1=t2[:])
                for h in range(HPC):          # v transposes (DMA xbar)
                    vst = st["vst"][h]
                    for q4 in range(PB // 128):
                        jt = pp * (PB // 128) + q4
                        eng = nc.sync if q4 % 2 == 0 else nc.scalar
                        eng.dma_start_transpose(
                            out=tiles["v"][h][:, jt, :],
                            in_=vst[:, q4 * 128:(q4 + 1) * 128])

            def proj_units(b):
                for pp in range(NPB):
                    for ft in range(FT):
                        yield ("grp", b, pp, ft)
                    yield ("fin", b, pp)

            def run_proj_unit(u):
                if u[0] == "grp":
                    emit_proj_group(u[1], u[2], u[3])
                else:
                    emit_proj_finalize(u[1], u[2])

            # ---------------- attention (primary stream) ----------------
            def emit_attn_batch(b, filler, op_q):
                """Emit attention+outproj for batch b, draining `filler`
                (an iterator of proj units for b+1) at a steady rate and
                `op_q` (outproj oc units) between jobs."""
                tiles = batch_tiles(b)
                q_t, k_t, v_t = tiles["q"], tiles["k"], tiles["v"]

                n_primary = 0
                for pp in range(NPB):
                    n_primary += (pp + 1) * (PB // JB) * HPC + 1
                n_filler = NPB * (FT + 1) if filler is not None else 0
                frate = n_filler / max(1, n_primary)
                fcredit = 0.0

                def tick():
                    nonlocal fcredit
                    fcredit += frate
                    while fcredit >= 1.0:
                        fcredit -= 1.0
                        u = next(filler, None)
                        if u is not None:
                            run_proj_unit(u)

                for pp in range(NPB):
                    nj = (pp + 1) * (PB // JB)
                    q0 = pp * PB
                    ytil = [ps.tile([128, PB], F32, tag="ytil", bufs=2,
                                    name=f"ytil{h}_{b}_{pp}")
                            for h in range(HPC)]
                    esum = [sb.tile([128, PB], F32, tag="esum", bufs=2,
                                    name=f"esum{h}_{b}_{pp}")
                            for h in range(HPC)]

                    def emit_S(h, j):
                        kk = j - pp * (PB // JB)
                        lo = max(kk, 0) * 128
                        sps = ps.tile([128, PB], F32, tag="ps512", bufs=6,
                                      name=f"s{h}_{b}_{pp}_{j}")
                        nc.tensor.matmul(
                            sps[:, lo:PB],
                            lhsT=k_t[h][:, j * JB:(j + 1) * JB],
                            rhs=q_t[h][:, q0 + lo:q0 + PB],
                            start=True, stop=True)
                        return sps

                    def emit_rest(h, j, sps):
                        kk = j - pp * (PB // JB)
                        lo = max(kk, 0) * 128
                        e = sb.tile([128, PB], BF, tag="e", bufs=4,
                                    name=f"e{h}_{b}_{pp}_{j}")
                        if kk >= 0:
                            # diag block: scale + causal mask over the whole
                            # valid range, then exp (scale already applied)
                            nc.vector.scalar_tensor_tensor(
                                out=sps[:, lo:PB],
                                in0=sps[:, lo:PB], scalar=scale,
                                in1=mask_sb[:, 384:384 + (PB - lo)],
                                op0=mybir.AluOpType.mult,
                                op1=mybir.AluOpType.add)
                            nc.scalar.activation(
                                out=e[:, lo:PB], in_=sps[:, lo:PB],
                                func=mybir.ActivationFunctionType.Exp)
                        else:
                            nc.scalar.activation(
                                out=e[:, lo:PB], in_=sps[:, lo:PB],
                                func=mybir.ActivationFunctionType.Exp,
                                scale=scale)
                        nc.tensor.matmul(ytil[h][:, lo:PB],
                                         lhsT=v_t[h][:, j, :],
                                         rhs=e[:, lo:PB], start=(j == 0),
                                         stop=(j == nj - 1))
                        # denominator accumulation on GpSimd (f32)
                        if j == 0:
                            nc.gpsimd.tensor_copy(out=esum[h][:, lo:PB],
                                                  in_=e[:, lo:PB])
                        else:
                            nc.gpsimd.tensor_tensor(
                                out=esum[h][:, lo:PB],
                                in0=esum[h][:, lo:PB], in1=e[:, lo:PB],
                                op=AluAdd)

                    jobs = [(h, j) for j in range(nj) for h in range(HPC)]
                    spss = {}
                    for idx in range(min(2, len(jobs))):
                        spss[jobs[idx]] = emit_S(*jobs[idx])
                    for idx, (h, j) in enumerate(jobs):
                        if op_q:
                            op_q.pop(0)()
                        if idx + 2 < len(jobs):
                            spss[jobs[idx + 2]] = emit_S(*jobs[idx + 2])
                        emit_rest(h, j, spss.pop((h, j)))
                        tick()

                    # ---- softmax finalize for panel pp ----
                    ypair = []
                    for h in range(HPC):
                        esb = sb.tile([128, PB], BF, tag="esb", bufs=2)
                        nc.vector.tensor_copy(out=esb[:], in_=esum[h][:])
                        den = ps.tile([128, PB], F32, tag="ps512", bufs=6)
                        nc.tensor.matmul(den[0:1, :], lhsT=ones_col[:],
                                         rhs=esb[:], start=True, stop=True)
                        dbf = sb.tile([1, PB], BF, tag="dbf", bufs=2)
                        nc.scalar.copy(out=dbf[:], in_=den[0:1, :])
                        bc = ps.tile([128, PB], F32, tag="ps512", bufs=6)
                        nc.tensor.matmul(bc[:], lhsT=ones_row[:],
                                         rhs=dbf[:], start=True, stop=True)
                        rec = sb.tile([128, PB], F32, tag="rec", bufs=2)
                        nc.vector.reciprocal_approx_fast(out=rec[:], in_=bc[:])
                        yp = sb.tile([128, PB], BF, tag="yp", bufs=4,
                                     name=f"yp{h}_{b}_{pp}")
                        nc.vector.tensor_mul(out=yp[:], in0=ytil[h][:],
                                             in1=rec[:])
                        ypair.append(yp)
                    tick()

                    # ---- queue outproj units for this panel ----
                    g = b * NPB + pp
                    zbig = sb.tile([128, NOC, PB], BF, tag="zbig", bufs=2,
                                   name=f"zbig_{b}_{pp}")

                    def mk_oc(oc, ypair=ypair, zbig=zbig):
                        def emit():
                            zps = ps.tile([128, PB], F32, tag="ps512", bufs=6)
                            for hh in range(HPC):
                                nc.tensor.matmul(
                                    zps[:],
                                    lhsT=wproj_sb[:, hh, oc * 128:(oc + 1) * 128],
                                    rhs=ypair[hh][:],
                                    start=(hh == 0), stop=(hh == HPC - 1))
                            nc.vector.tensor_copy(out=zbig[:, oc, :],
                                                  in_=zps[:])
                        return emit

                    def mk_dma(g=g, zbig=zbig):
                        def emit():
                            nc.sync.dma_start(out=zout[g], in_=zbig[:])
                        return emit

                    for oc in range(NOC):
                        op_q.append(mk_oc(oc))
                    op_q.append(mk_dma())

                # flush filler at end of batch slot
                while True:
                    u = next(filler, None) if filler is not None else None
                    if u is None:
                        break
                    run_proj_unit(u)

            # ---------------- top-level schedule ----------------
            load_xt(0)
            for u in proj_units(0):       # batch 0 projection, unwoven
                run_proj_unit(u)
            op_q = []
            for b in range(B):
                filler = iter(proj_units(b + 1)) if b + 1 < B else None
                emit_attn_batch(b, filler, op_q)
            while op_q:                   # tail outproj units
                op_q.pop(0)()

    nc.compile()
    return nc


_module_cache = {}


def _get_module(B, T):
    key = (B, T)
    if key not in _module_cache:
        _module_cache[key] = build_module(B, T)
    return _module_cache[key]


def _host_prep(x, Wqkv, Wproj, B, T):
    bf16 = ml_dtypes.bfloat16
    BT = B * T
    NP = BT // PB
    CC = C // 128
    x2 = x.reshape(NP, PB, CC, 128)
    xtiles = np.ascontiguousarray(
        x2.transpose(0, 3, 2, 1).reshape(NP, 128, CC * PB)).astype(bf16)

    inv = 1.0 / (ROPE_BASE ** (np.arange(0, D, 2, dtype=np.float32) / D))
    t = np.arange(T, dtype=np.float32)
    fr = np.outer(t, inv)                      # [T, 64]
    emb = np.concatenate([fr, fr], -1)         # [T, 128]
    cosT = np.ascontiguousarray(np.cos(emb).T).astype(bf16)
    sinT = np.ascontiguousarray(np.sin(emb).T).astype(np.float32)

    g = np.arange(896)[None, :]
    p = np.arange(128)[:, None]
    maskT = np.where(g >= p + 384, 0.0, NEG).astype(np.float32)

    permT = np.zeros((128, 128), np.float32)
    for j in range(64):
        permT[j, j + 64] = 1.0                 # rot[i] = q[i-64] for i>=64
    for j in range(64, 128):
        permT[j, j - 64] = -1.0                # rot[i] = -q[i+64] for i<64
    permT = permT.astype(bf16)

    in_maps = []
    for c in range(N_CORES):
        heads = [HPC * c + h for h in range(HPC)]
        rows = []
        for blk in range(3):                   # q, k, v blocks of Wqkv
            for h in heads:
                r0 = blk * C + h * D
                rows.append(Wqkv[r0:r0 + D])
        wslice = np.concatenate(rows, 0)       # [FT*128, C]
        wqkvT = np.ascontiguousarray(wslice.T).astype(bf16)
        cols = np.concatenate([np.arange(h * D, (h + 1) * D) for h in heads])
        wprojT = np.ascontiguousarray(Wproj[:, cols].T).astype(bf16)
        in_maps.append({
            "xtiles": xtiles,
            "wqkvT": wqkvT,
            "wprojT": wprojT,
            "cosT": cosT,
            "sinT": sinT,
            "maskT": maskT,
            "permT": permT,
        })
    return in_maps


last_results = None


def kernel(x, Wqkv, Wproj, _trace=False, _trace_kwargs=None):
    global last_results
    x = np.asarray(x, dtype=np.float32)
    Wqkv = np.asarray(Wqkv, dtype=np.float32)
    Wproj = np.asarray(Wproj, dtype=np.float32)
    B, T, _C = x.shape
    assert _C == C and T % PB == 0

    nc = _get_module(B, T)
    in_maps = _host_prep(x, Wqkv, Wproj, B, T)
    res = run_bass_kernel_spmd(nc, in_maps, core_ids=list(range(N_CORES)),
                               trace=_trace, **(_trace_kwargs or {}))
    last_results = res
    NOC = C // 128
    NP = (B * T) // PB
    z = res.results[0]["zout"].astype(np.float32)
    for c in range(1, N_CORES):
        z += res.results[c]["zout"].astype(np.float32)
    # zout[g, p, oc, r] = z[oc*128+p, g*PB+r];  y[t, c] = z[c, t]
    y = z.transpose(0, 3, 2, 1).reshape(B, T, C)
    return y


# revision 12
# speedup vs baseline: 1.4885x; 1.1926x over previous
"""Causal self-attention (QKV proj + RoPE + causal SDPA + out proj) on 8 trn2 cores.

Sharding: tensor-parallel over heads. Each core owns 2 of 16 heads:
  - Wqkv column-split (the core's q/k/v head rows), Wproj row-split.
  - Each core computes a full-shape partial of the output projection;
    the 8 partials are summed (and transposed back) on the host.

Device-side layout: everything runs transposed (x^T fed as [C, B*T];
qkv^T = W @ x keeps head dims on partitions). v is transposed on-chip
with DMA xbar transposes.

Schedule: batch b's attention+outproj is woven with batch b+1's
projection so the tensor engine never starves on the exp (scalar
engine) dependency chain. Softmax denominators are accumulated on
GpSimd (f32) instead of per-tile ones-matmuls on the PE.
"""
import sys

sys.path.insert(0, "/opt/trn_rl_repo")

import numpy as np
import ml_dtypes

import concourse.bacc as bacc
import concourse.mybir as mybir
import concourse.tile as tile
from concourse.bass_utils import run_bass_kernel_spmd

N_CORES = 8
C = 2048
H = 16
D = 128
HPC = H // N_CORES          # heads per core = 2
PB = 512                    # row panel width
JB = 128                    # key tile width
NEG = -1.0e30
ROPE_BASE = 10000.0

BF = mybir.dt.bfloat16
F16 = mybir.dt.float16
F32 = mybir.dt.float32


def build_module(B, T):
    BT = B * T
    CC = C // 128            # contraction chunks for the projection
    FT = 3 * HPC             # qkv f-tiles per core (q0 q1 k0 k1 v0 v1)
    NPB = T // PB            # panels per batch
    NOC = C // 128           # out-proj column tiles
    NPANEL = B * NPB
    scale = 1.0 / float(np.sqrt(D))
    AluAdd = mybir.AluOpType.add

    nc = bacc.Bacc("TRN2", target_bir_lowering=False, debug=False,
                   num_devices=N_CORES)

    # x pre-tiled on host: xtiles[g, p, cc*PB + r] = x[g*PB + r, cc*128 + p]
    xtiles = nc.dram_tensor("xtiles", [NPANEL, 128, CC * PB], BF,
                            kind="ExternalInput").ap()
    wqkvT = nc.dram_tensor("wqkvT", [C, FT * 128], BF, kind="ExternalInput").ap()
    wprojT = nc.dram_tensor("wprojT", [HPC * 128, C], BF, kind="ExternalInput").ap()
    cosT = nc.dram_tensor("cosT", [128, T], BF, kind="ExternalInput").ap()
    sinT = nc.dram_tensor("sinT", [128, T], BF, kind="ExternalInput").ap()
    maskT = nc.dram_tensor("maskT", [128, 896], F32, kind="ExternalInput").ap()
    permT = nc.dram_tensor("permT", [128, 128], BF, kind="ExternalInput").ap()
    # tiled output: zout[g, p, oc, r] = z[oc*128 + p, g*PB + r]
    # -> each panel store is one fully-contiguous 2MB DMA (16KB runs)
    zout = nc.dram_tensor("zout", [NPANEL, 128, NOC, PB], BF,
                          kind="ExternalOutput").ap()

    with tile.TileContext(nc) as tc:
        with tc.tile_pool(name="sb", bufs=1) as sb, \
             tc.tile_pool(name="ps", bufs=1, space="PSUM") as ps:
            # ---- resident constants ----
            wqkv_sb = sb.tile([128, CC, FT * 128], BF, tag="wqkv", bufs=1)
            nc.sync.dma_start(
                out=wqkv_sb[:],
                in_=wqkvT.rearrange("(cc p) f -> p cc f", p=128))
            wproj_sb = sb.tile([128, HPC, C], BF, tag="wproj", bufs=1)
            nc.sync.dma_start(
                out=wproj_sb[:],
                in_=wprojT.rearrange("(hh p) o -> p hh o", p=128))
            cos_sb = sb.tile([128, T], BF, tag="cos", bufs=1)
            nc.sync.dma_start(out=cos_sb[:], in_=cosT)
            sin_sb = sb.tile([128, T], BF, tag="sin", bufs=1)
            nc.sync.dma_start(out=sin_sb[:], in_=sinT)
            mask_sb = sb.tile([128, 896], F32, tag="mask", bufs=1)
            nc.sync.dma_start(out=mask_sb[:], in_=maskT)
            perm_sb = sb.tile([128, 128], BF, tag="perm", bufs=1)
            nc.sync.dma_start(out=perm_sb[:], in_=permT)
            ones_col = sb.tile([128, 1], F16, tag="ones_c", bufs=1)
            nc.vector.memset(ones_col[:], 1.0)
            ones_row = sb.tile([1, 128], BF, tag="ones_r", bufs=1)
            nc.vector.memset(ones_row[:], 1.0)

            HC = CC // 2
            xt_map = {}          # panel g -> (xta, xtb)
            bt_map = {}          # batch b -> dict(q=,k=,v=)

            def load_xt(g):
                xta = sb.tile([128, HC, PB], BF, tag="xta", bufs=2,
                              name=f"xta_{g}")
                xtb = sb.tile([128, HC, PB], BF, tag="xtb", bufs=2,
                              name=f"xtb_{g}")
                src = xtiles[g].rearrange("p (cc r) -> p cc r", r=PB)
                nc.sync.dma_start(out=xta[:], in_=src[:, :HC, :])
                nc.gpsimd.dma_start(out=xtb[:], in_=src[:, HC:, :])
                xt_map[g] = (xta, xtb)

            def batch_tiles(b):
                if b not in bt_map:
                    bt_map[b] = {
                        "q": [sb.tile([128, T], BF, tag=f"q{h}", bufs=2,
                                      name=f"q{h}_{b}") for h in range(HPC)],
                        "k": [sb.tile([128, T], BF, tag=f"k{h}", bufs=2,
                                      name=f"k{h}_{b}") for h in range(HPC)],
                        "v": [sb.tile([128, T // 128, 128], BF, tag=f"v{h}",
                                      bufs=2, name=f"v{h}_{b}")
                              for h in range(HPC)],
                    }
                return bt_map[b]

            # ---------------- projection (filler stream) ----------------
            # per panel: pending raw/vst tiles for the finalize unit
            proj_state = {}

            def emit_proj_group(b, pp, ft):
                g = b * NPB + pp
                if ft == 0:
                    if g + 1 < NPANEL:
                        load_xt(g + 1)
                    proj_state[(b, pp)] = {"raw": {}, "vst": {}}
                xt = xt_map[g]
                pps = ps.tile([128, PB], F32, tag="ps512", bufs=6)
                for cc in range(CC):
                    xsrc = xt[0][:, cc, :] if cc < HC else xt[1][:, cc - HC, :]
                    nc.tensor.matmul(
                        pps[:],
                        lhsT=wqkv_sb[:, cc, ft * 128:(ft + 1) * 128],
                        rhs=xsrc,
                        start=(cc == 0), stop=(cc == CC - 1))
                st = proj_state[(b, pp)]
                if ft < 2 * HPC:       # q or k: stage raw for rope
                    raw = sb.tile([128, PB], BF, tag="qkraw", bufs=4,
                                  name=f"raw_{b}_{pp}_{ft}")
                    nc.scalar.copy(out=raw[:], in_=pps[:])
                    st["raw"][ft] = raw
                else:                  # v: stage for dma-transpose
                    vst = sb.tile([128, PB], BF, tag="vstage", bufs=4,
                                  name=f"vst_{b}_{pp}_{ft}")
                    nc.scalar.copy(out=vst[:], in_=pps[:])
                    st["vst"][ft - 2 * HPC] = vst

            def emit_proj_finalize(b, pp):
                ts = slice(pp * PB, pp * PB + PB)
                tiles = batch_tiles(b)
                st = proj_state.pop((b, pp))
                rots = {}
                for ft in range(2 * HPC):     # rope rotate matmuls
                    rot = ps.tile([128, PB], F32, tag="ps512", bufs=6)
                    nc.tensor.matmul(rot[:], lhsT=perm_sb[:],
                                     rhs=st["raw"][ft][:], start=True,
                                     stop=True)
                    rotb = sb.tile([128, PB], BF, tag="rotb", bufs=4,
                                   name=f"rotb_{b}_{pp}_{ft}")
                    nc.scalar.copy(out=rotb[:], in_=rot[:])
                    rots[ft] = rotb
                for ft in range(2 * HPC):     # rope elementwise on DVE (2x)
                    raw = st["raw"][ft]
                    t1 = sb.tile([128, PB], BF, tag="t1", bufs=2)
                    nc.vector.tensor_mul(out=t1[:], in0=raw[:],
                                         in1=cos_sb[:, ts])
                    t2 = sb.tile([128, PB], BF, tag="t2", bufs=2)
                    nc.vector.tensor_mul(out=t2[:], in0=rots[ft][:],
                                         in1=sin_sb[:, ts])
                    dest = (tiles["q"] if ft < HPC else tiles["k"])[ft % HPC]
                    nc.vector.tensor_add(out=dest[:, ts], in0=t1[:],
                                         in1=t2[:])
                for h in range(HPC):          # v transposes (DMA xbar)
                    vst = st["vst"][h]
                    for q4 in range(PB // 128):
                        jt = pp * (PB // 128) + q4
                        nc.sync.dma_start_transpose(
                            out=tiles["v"][h][:, jt, :],
                            in_=vst[:, q4 * 128:(q4 + 1) * 128])

            def proj_units(b):
                for pp in range(NPB):
                    for ft in range(FT):
                        yield ("grp", b, pp, ft)
                    yield ("fin", b, pp)

            def run_proj_unit(u):
                if u[0] == "grp":
                    emit_proj_group(u[1], u[2], u[3])
                else:
                    emit_proj_finalize(u[1], u[2])

            # ---------------- attention (primary stream) ----------------
            def emit_attn_batch(b, filler, op_q):
                """Emit attention+outproj for batch b, draining `filler`
                (an iterator of proj units for b+1) at a steady rate and
                `op_q` (outproj oc units) between jobs."""
                tiles = batch_tiles(b)
                q_t, k_t, v_t = tiles["q"], tiles["k"], tiles["v"]

                n_primary = 0
                for pp in range(NPB):
                    n_primary += (pp + 1) * (PB // JB) * HPC + 1
                n_filler = NPB * (FT + 1) if filler is not None else 0
                frate = n_filler / max(1, n_primary)
                fcredit = 0.0

                def tick():
                    nonlocal fcredit
                    fcredit += frate
                    while fcredit >= 1.0:
                        fcredit -= 1.0
                        u = next(filler, None)
                        if u is not None:
                            run_proj_unit(u)

                for pp in range(NPB):
                    nj = (pp + 1) * (PB // JB)
                    q0 = pp * PB
                    ytil = [ps.tile([128, PB], F32, tag="ytil", bufs=2,
                                    name=f"ytil{h}_{b}_{pp}")
                            for h in range(HPC)]
                    # two bf16 partial-sum accumulators per head (DVE),
                    # combined by two accumulating ones-matmuls later
                    esum = [[sb.tile([128, PB], F16, tag="esum", bufs=8,
                                     name=f"esum{h}{par}_{b}_{pp}")
                             for par in range(2)] for h in range(HPC)]
                    blo = [0, 0]   # valid-from column of partial 0/1

                    def emit_S(h, j):
                        kk = j - pp * (PB // JB)
                        lo = max(kk, 0) * 128
                        sps = ps.tile([128, PB], F32, tag="ps512", bufs=6,
                                      name=f"s{h}_{b}_{pp}_{j}")
                        nc.tensor.matmul(
                            sps[:, lo:PB],
                            lhsT=k_t[h][:, j * JB:(j + 1) * JB],
                            rhs=q_t[h][:, q0 + lo:q0 + PB],
                            start=True, stop=True)
                        return sps

                    def emit_rest(h, j, sps):
                        kk = j - pp * (PB // JB)
                        lo = max(kk, 0) * 128
                        e = sb.tile([128, PB], F16, tag="e", bufs=4,
                                    name=f"e{h}_{b}_{pp}_{j}")
                        if kk >= 0:
                            # diag block: scale + causal mask over the whole
                            # valid range, then exp (scale already applied)
                            nc.vector.scalar_tensor_tensor(
                                out=sps[:, lo:PB],
                                in0=sps[:, lo:PB], scalar=scale,
                                in1=mask_sb[:, 384:384 + (PB - lo)],
                                op0=mybir.AluOpType.mult,
                                op1=mybir.AluOpType.add)
                            nc.scalar.activation(
                                out=e[:, lo:PB], in_=sps[:, lo:PB],
                                func=mybir.ActivationFunctionType.Exp)
                        else:
                            nc.scalar.activation(
                                out=e[:, lo:PB], in_=sps[:, lo:PB],
                                func=mybir.ActivationFunctionType.Exp,
                                scale=scale)
                        nc.tensor.matmul(ytil[h][:, lo:PB],
                                         lhsT=v_t[h][:, j, :],
                                         rhs=e[:, lo:PB], start=(j == 0),
                                         stop=(j == nj - 1))
                        # denominator partial accumulation (DVE, bf16 2x)
                        acc = esum[h][j % 2]
                        if j < 2:
                            blo[j] = lo
                            nc.vector.tensor_copy(out=acc[:, lo:PB],
                                                  in_=e[:, lo:PB])
                        else:
                            nc.vector.tensor_tensor(
                                out=acc[:, lo:PB],
                                in0=acc[:, lo:PB], in1=e[:, lo:PB],
                                op=AluAdd)

                    jobs = [(h, j) for h in range(HPC) for j in range(nj)]
                    spss = {}
                    for idx in range(min(2, len(jobs))):
                        spss[jobs[idx]] = emit_S(*jobs[idx])
                    for idx, (h, j) in enumerate(jobs):
                        if op_q:
                            op_q.pop(0)()
                        if idx + 2 < len(jobs):
                            spss[jobs[idx + 2]] = emit_S(*jobs[idx + 2])
                        emit_rest(h, j, spss.pop((h, j)))
                        tick()

                    # ---- deferred softmax finalize + outproj units ----
                    g = b * NPB + pp
                    b0, b1 = blo[0], blo[1]
                    zbig = sb.tile([128, NOC, PB], BF, tag="zbig", bufs=2,
                                   name=f"zbig_{b}_{pp}")
                    ypair = [sb.tile([128, PB], BF, tag="yp", bufs=4,
                                     name=f"yp{h}_{b}_{pp}")
                             for h in range(HPC)]

                    def mk_den(h, es=esum, yt=ytil, yps=ypair, b0=b0, b1=b1):
                        def emit():
                            den = ps.tile([128, PB], F32, tag="ps512", bufs=6)
                            nc.tensor.matmul(den[0:1, b0:PB],
                                             lhsT=ones_col[:],
                                             rhs=es[h][0][:, b0:PB],
                                             start=True, stop=False)
                            nc.tensor.matmul(den[0:1, b1:PB],
                                             lhsT=ones_col[:],
                                             rhs=es[h][1][:, b1:PB],
                                             start=False, stop=True)
                            dbf = sb.tile([1, PB], BF, tag="dbf", bufs=2)
                            nc.scalar.copy(out=dbf[:], in_=den[0:1, :])
                            bc = ps.tile([128, PB], F32, tag="ps512", bufs=6)
                            nc.tensor.matmul(bc[:], lhsT=ones_row[:],
                                             rhs=dbf[:], start=True,
                                             stop=True)
                            rec = sb.tile([128, PB], F32, tag="rec", bufs=2)
                            nc.vector.reciprocal_approx_fast(out=rec[:],
                                                             in_=bc[:])
                            nc.vector.tensor_mul(out=yps[h][:],
                                                 in0=yt[h][:], in1=rec[:])
                        return emit

                    def mk_oc(oc, yps=ypair, zbig=zbig):
                        def emit():
                            zps = ps.tile([128, PB], F32, tag="ps512", bufs=6)
                            for hh in range(HPC):
                                nc.tensor.matmul(
                                    zps[:],
                                    lhsT=wproj_sb[:, hh, oc * 128:(oc + 1) * 128],
                                    rhs=yps[hh][:],
                                    start=(hh == 0), stop=(hh == HPC - 1))
                            nc.vector.tensor_copy(out=zbig[:, oc, :],
                                                  in_=zps[:])
                        return emit

                    def mk_dma(g=g, zbig=zbig):
                        def emit():
                            nc.sync.dma_start(out=zout[g], in_=zbig[:])
                        return emit

                    # den/yp units MUST drain before the next panel's first
                    # AV matmuls (they release the ytil PSUM banks), so they
                    # go at the front of the deferred queue
                    op_q[0:0] = [mk_den(h) for h in range(HPC)]
                    for oc in range(NOC):
                        op_q.append(mk_oc(oc))
                    op_q.append(mk_dma())

                # flush filler at end of batch slot
                while True:
                    u = next(filler, None) if filler is not None else None
                    if u is None:
                        break
                    run_proj_unit(u)

            # ---------------- top-level schedule ----------------
            load_xt(0)
            for u in proj_units(0):       # batch 0 projection, unwoven
                run_proj_unit(u)
            op_q = []
            for b in range(B):
                filler = iter(proj_units(b + 1)) if b + 1 < B else None
                emit_attn_batch(b, filler, op_q)
            while op_q:                   # tail outproj units
                op_q.pop(0)()

    nc.compile()
    return nc


_module_cache = {}


def _get_module(B, T):
    key = (B, T)
    if key not in _module_cache:
        _module_cache[key] = build_module(B, T)
    return _module_cache[key]


def _host_prep(x, Wqkv, Wproj, B, T):
    bf16 = ml_dtypes.bfloat16
    BT = B * T
    NP = BT // PB
    CC = C // 128
    x2 = x.reshape(NP, PB, CC, 128)
    xtiles = np.ascontiguousarray(
        x2.transpose(0, 3, 2, 1).reshape(NP, 128, CC * PB)).astype(bf16)

    inv = 1.0 / (ROPE_BASE ** (np.arange(0, D, 2, dtype=np.float32) / D))
    t = np.arange(T, dtype=np.float32)
    fr = np.outer(t, inv)                      # [T, 64]
    emb = np.concatenate([fr, fr], -1)         # [T, 128]
    cosT = np.ascontiguousarray(np.cos(emb).T).astype(bf16)
    sinT = np.ascontiguousarray(np.sin(emb).T).astype(bf16)

    g = np.arange(896)[None, :]
    p = np.arange(128)[:, None]
    maskT = np.where(g >= p + 384, 0.0, NEG).astype(np.float32)

    permT = np.zeros((128, 128), np.float32)
    for j in range(64):
        permT[j, j + 64] = 1.0                 # rot[i] = q[i-64] for i>=64
    for j in range(64, 128):
        permT[j, j - 64] = -1.0                # rot[i] = -q[i+64] for i<64
    permT = permT.astype(bf16)

    in_maps = []
    for c in range(N_CORES):
        heads = [HPC * c + h for h in range(HPC)]
        rows = []
        for blk in range(3):                   # q, k, v blocks of Wqkv
            for h in heads:
                r0 = blk * C + h * D
                rows.append(Wqkv[r0:r0 + D])
        wslice = np.concatenate(rows, 0)       # [FT*128, C]
        wqkvT = np.ascontiguousarray(wslice.T).astype(bf16)
        cols = np.concatenate([np.arange(h * D, (h + 1) * D) for h in heads])
        wprojT = np.ascontiguousarray(Wproj[:, cols].T).astype(bf16)
        in_maps.append({
            "xtiles": xtiles,
            "wqkvT": wqkvT,
            "wprojT": wprojT,
            "cosT": cosT,
            "sinT": sinT,
            "maskT": maskT,
            "permT": permT,
        })
    return in_maps


last_results = None


def kernel(x, Wqkv, Wproj, _trace=False, _trace_kwargs=None):
    global last_results
    x = np.asarray(x, dtype=np.float32)
    Wqkv = np.asarray(Wqkv, dtype=np.float32)
    Wproj = np.asarray(Wproj, dtype=np.float32)
    B, T, _C = x.shape
    assert _C == C and T % PB == 0

    nc = _get_module(B, T)
    in_maps = _host_prep(x, Wqkv, Wproj, B, T)
    res = run_bass_kernel_spmd(nc, in_maps, core_ids=list(range(N_CORES)),
                               trace=_trace, **(_trace_kwargs or {}))
    last_results = res
    NOC = C // 128
    NP = (B * T) // PB
    z = res.results[0]["zout"].astype(np.float32)
    for c in range(1, N_CORES):
        z += res.results[c]["zout"].astype(np.float32)
    # zout[g, p, oc, r] = z[oc*128+p, g*PB+r];  y[t, c] = z[c, t]
    y = z.transpose(0, 3, 2, 1).reshape(B, T, C)
    return y
